# revision 1
# baseline (speedup 1.0000x reference)
"""Local2d (locally-connected conv, unshared weights) Trainium2 kernel.

Problem: out[b,o,h,w] = sum_{i,k,l} weight[o,h,w,i,k,l] * xpad[b,i,h+k,w+l] + bias[o,h,w]
  x: [64, 64, 32, 32] f32, weight: [128, 32, 32, 64, 3, 3] f32, bias: [128, 32, 32] f32
  out: [64, 128, 32, 32] f32

Strategy: shard the 32 output rows h across 8 cores (4 rows each). Each output
location (h,w) is an independent GEMM: [o=128] x [ikl=576] @ [ikl=576] x [b=64],
chunked as 3 K=128 matmuls (taps k in {0,1} paired with equal l on the partition
dim) plus 3 K=64 matmuls (k=2), PSUM-accumulated. Host ships fp16 weights in
[h, l, (k i), w, o] layout and raw padded x-row windows (1 DMA each, partition
dim always a single source axis — merged-source partition DMAs crash the
device). On-chip, DVE expands each x-row window into patch-shaped tiles with 3
shifted copies, so matmuls read non-overlapping slices (clean Tile dependency
graph; reading overlapping w+l windows directly from the row tile is 3x slower).
DVE also applies bias from PSUM into an fp16 [o, w, b] output tile, written
back once per row and reassembled/transposed on host. ~24.3MB DMA per core at
~390GB/s => ~62us, HBM-bound; fp16 inputs give rel err ~3.6e-4 vs the fp32
reference.
"""

import os
import numpy as np

B, C_IN, C_OUT, KS, H, W = 64, 64, 128, 3, 32, 32
H_OUT, W_OUT = 32, 32
N_CORES = 8
H_PER = H_OUT // N_CORES  # 4
IKL = C_IN * KS * KS  # 576
NCHUNK = 5
IKLP = NCHUNK * 128  # 640, ikl zero-padded so every chunk is K=128 (FWL-eligible)

_NC_CACHE = {}
_RUNNER_CACHE = {}
_LAST_IN_MAPS = None
LAST_RESULT = None


def _split_multiwaits(nc):
    """This container's walrus accepts at most ONE sync-wait per instruction.
    Hoist extra waits onto single-wait NoOps on the same engine, inserted
    immediately before (engine streams are in-order, sem waits are >=-monotonic,
    so this is semantics-preserving)."""
    import concourse.mybir as mybir

    ctr = 0
    hist = {}
    for f in nc.m.functions:
        for blk in f.blocks:
            insts = list(blk.instructions)
            changed = False
            newlist = []
            for inst in insts:
                si = inst.sync_info
                if si is not None and si.on_wait and len(si.on_wait) > 1:
                    tname = type(inst).__name__
                    hist[tname] = hist.get(tname, 0) + 1
                    waits = list(si.on_wait)
                    for wt in waits[:-1]:
                        nop = mybir.InstNoOp(name=f"splitwait-{ctr}", ins=[], outs=[])
                        ctr += 1
                        nop.engine = inst.engine
                        nop.sync_info = mybir.SyncInfo(on_wait=[wt], on_update=[])
                        newlist.append(nop)
                    inst.sync_info = mybir.SyncInfo(
                        on_wait=[waits[-1]], on_update=list(si.on_update or [])
                    )
                    changed = True
                newlist.append(inst)
            if changed:
                blk.instructions = newlist
    if os.environ.get("K_DEBUG"):
        print(f"split_multiwaits: {ctr} extra waits hoisted; by type: {hist}")
    return ctr


def _build_nc(dt_name, reps=1):
    import concourse.bass as bass
    import concourse.mybir as mybir
    import concourse.tile as tile

    dt_in = getattr(mybir.dt, dt_name)
    nc = bass.Bass()
    # Tap-paired scheme: chunks pair taps with EQUAL l and k in {0,1} on the
    # partition dim (both halves then read the same free offset w+l), plus a
    # K=64 chunk for k=2 — raw x row windows serve as rhs directly, no patch
    # materialization. Host pre-merges (k,i)->ki and pre-windows x rows so
    # every DMA partition dim is a single source axis.
    wm_d = nc.dram_tensor(
        "wm", [H_PER, KS, 2 * C_IN, W_OUT, C_OUT], dt_in, kind="ExternalInput"
    )
    w2_d = nc.dram_tensor(
        "w2", [H_PER, KS, C_IN, W_OUT, C_OUT], dt_in, kind="ExternalInput"
    )
    pm_d = nc.dram_tensor(
        "pm", [H_PER, 2 * C_IN, W + 2, B], dt_in, kind="ExternalInput"
    )
    p2_d = nc.dram_tensor(
        "p2", [H_PER, C_IN, W + 2, B], dt_in, kind="ExternalInput"
    )
    b_d = nc.dram_tensor(
        "bias", [C_OUT, H_PER, W_OUT], mybir.dt.float32, kind="ExternalInput"
    )
    o_d = nc.dram_tensor(
        "out", [C_OUT, H_PER, W_OUT, B], mybir.dt.float16, kind="ExternalOutput"
    )

    with tile.TileContext(nc) as tc:
        with (
            tc.tile_pool(name="wp", bufs=2) as wp,
            tc.tile_pool(name="pp", bufs=2) as pp,
            tc.tile_pool(name="op", bufs=2) as op,
            tc.tile_pool(name="bp", bufs=1) as bp,
            tc.tile_pool(name="psp", bufs=8, space="PSUM") as psp,
        ):
            bias_sb = bp.tile([C_OUT, H_PER, W_OUT], mybir.dt.float32, name="bias_sb")
            nc.gpsimd.dma_start(bias_sb[:], b_d[:])
            for rep in range(reps):
                for h in range(H_PER):
                    # alternate the two HWDGE rings between the big streams
                    weng = nc.sync if h % 2 == 0 else nc.scalar
                    peng = nc.scalar if h % 2 == 0 else nc.sync
                    wm = wp.tile(
                        [128, KS, W_OUT, C_OUT], dt_in, tag="wm", name=f"wm_{rep}_{h}"
                    )
                    weng.dma_start(
                        wm[:], wm_d[h].rearrange("l p w o -> p l w o")
                    )
                    w2 = wp.tile(
                        [C_IN, KS, W_OUT, C_OUT], dt_in, tag="w2", name=f"w2_{rep}_{h}"
                    )
                    weng.dma_start(w2[:], w2_d[h].rearrange("l p w o -> p l w o"))
                    t01 = pp.tile(
                        [128, W + 2, B], dt_in, tag="t01", name=f"t01_{rep}_{h}"
                    )
                    peng.dma_start(t01[:], pm_d[h])
                    t2 = pp.tile(
                        [C_IN, W + 2, B], dt_in, tag="t2", name=f"t2_{rep}_{h}"
                    )
                    peng.dma_start(t2[:], p2_d[h])
                    # expand x-row windows into patch-shaped tiles on-chip
                    # (within-partition shifted copies) so matmuls read
                    # non-overlapping slices — keeps the dependency graph
                    # v3-shaped while x rows travel over DMA only once per h.
                    pl01 = pp.tile(
                        [128, KS, W_OUT, B], dt_in, tag="pl01",
                        name=f"pl01_{rep}_{h}",
                    )
                    pl2 = pp.tile(
                        [C_IN, KS, W_OUT, B], dt_in, tag="pl2",
                        name=f"pl2_{rep}_{h}",
                    )
                    for l in range(KS):
                        nc.vector.tensor_copy(
                            pl01[:, l, :, :], t01[:, l : l + W_OUT, :]
                        )
                        nc.vector.tensor_copy(
                            pl2[:, l, :, :], t2[:, l : l + W_OUT, :]
                        )
                    ot = op.tile(
                        [C_OUT, W_OUT, B], mybir.dt.float16, tag="ot",
                        name=f"ot_{rep}_{h}",
                    )
                    for w in range(W_OUT):
                        ps = psp.tile(
                            [C_OUT, B], mybir.dt.float32, tag="ps",
                            name=f"ps_{rep}_{h}_{w}",
                        )
                        for l in range(KS):
                            nc.tensor.matmul(
                                ps[:],
                                wm[:, l, w, :],
                                pl01[:, l, w, :],
                                start=(l == 0),
                                stop=False,
                            )
                        for l in range(KS):
                            nc.tensor.matmul(
                                ps[:],
                                w2[:, l, w, :],
                                pl2[:, l, w, :],
                                start=False,
                                stop=(l == KS - 1),
                            )
                        nc.vector.tensor_scalar_add(
                            ot[:, w, :], ps[:], bias_sb[:, h, w : w + 1]
                        )
                    nc.gpsimd.dma_start(o_d[:, h], ot[:])

    _split_multiwaits(nc)
    return nc


def _get_nc(dt_name, reps=1):
    key = (dt_name, reps)
    if key not in _NC_CACHE:
        _NC_CACHE[key] = _build_nc(dt_name, reps)
    return _NC_CACHE[key]


def _prepare_in_maps(x, weight, bias, dt_np):
    x = np.asarray(x, dtype=np.float32)
    weight = np.asarray(weight, dtype=np.float32)
    bias = np.asarray(bias, dtype=np.float32)

    # padded x rows [h'=34, i, w'=34, b]
    x_t = np.zeros((H + 2, C_IN, W + 2, B), dtype=dt_np)
    x_t[1 : H + 1, :, 1 : W + 1, :] = x.transpose(2, 1, 3, 0)

    # weight -> [h, l, k, i, w, o]
    Wt = weight.transpose(1, 5, 4, 3, 2, 0).astype(dt_np)

    in_maps = []
    for c in range(N_CORES):
        h0 = c * H_PER
        wc = Wt[h0 : h0 + H_PER]  # [4, l, k, i, w, o]
        wm = np.ascontiguousarray(wc[:, :, 0:2]).reshape(
            H_PER, KS, 2 * C_IN, W_OUT, C_OUT
        )
        w2 = np.ascontiguousarray(wc[:, :, 2])
        # x row windows: pm[h] = rows (h0+h, h0+h+1) stacked on (k i); p2[h] = row h0+h+2
        pm = np.stack(
            [
                x_t[h0 + h : h0 + h + 2].reshape(2 * C_IN, W + 2, B)
                for h in range(H_PER)
            ]
        )
        p2 = np.ascontiguousarray(x_t[h0 + 2 : h0 + 2 + H_PER])
        in_maps.append(
            {
                "wm": wm,
                "w2": w2,
                "pm": pm,
                "p2": p2,
                "bias": np.ascontiguousarray(bias[:, h0 : h0 + H_PER, :]),
            }
        )
    return in_maps


def kernel(x, weight, bias):
    global _LAST_IN_MAPS

    dt_name = os.environ.get("K_DTYPE", "float16")
    dt_np = {"float16": np.float16, "float32": np.float32}[dt_name]

    in_maps = _prepare_in_maps(x, weight, bias, dt_np)
    _LAST_IN_MAPS = in_maps

    fn, in_names, zero_outs, sharding = _get_runner(dt_name, 1)
    concat_in, concat_zero = _stage(
        dt_name, in_maps, in_names, zero_outs, sharding, fresh=True
    )
    outs = fn(*concat_in, *concat_zero)
    out_global = np.asarray(outs[0])  # (8*128, H_PER, 32, 64) fp16

    out = np.concatenate(
        [out_global[c * C_OUT : (c + 1) * C_OUT] for c in range(N_CORES)], axis=1
    )  # [o, 32, 32, b]
    return np.ascontiguousarray(
        out.transpose(3, 0, 1, 2).astype(np.float32)
    )


# ---------------------------------------------------------------------------
# Timing (NTFF profiling is unavailable in this container: antenv.axon_hooks
# missing). Measure differentially instead: jit the NEFF exec for reps=1 and
# reps=R bodies, pre-stage inputs on devices, time N pipelined executions of
# each, and report (T_R - T_1) / (N * (R - 1)).
# ---------------------------------------------------------------------------


def _make_runner(nc):
    import jax
    import concourse.mybir as mybir
    from concourse.bass2jax import (
        _bass_exec_p,
        install_neuronx_cc_hook,
        partition_id_tensor,
    )
    from jax.experimental.shard_map import shard_map
    from jax.sharding import Mesh, NamedSharding, PartitionSpec

    install_neuronx_cc_hook()

    partition_name = nc.partition_id_tensor.name if nc.partition_id_tensor else None
    in_names, out_names, out_avals, zero_outs = [], [], [], []
    for alloc in nc.m.functions[0].allocations:
        if not isinstance(alloc, mybir.MemoryLocationSet):
            continue
        name = alloc.memorylocations[0].name
        if alloc.kind == "ExternalInput":
            if name != partition_name:
                in_names.append(name)
        elif alloc.kind == "ExternalOutput":
            out_names.append(name)
            shape = tuple(alloc.tensor_shape)
            dtype = mybir.dt.np(alloc.dtype)
            out_avals.append(jax.core.ShapedArray(shape, dtype))
            zero_outs.append(np.zeros(shape, dtype))
    n_params = len(in_names)
    all_names = in_names + out_names
    if partition_name is not None:
        all_names = all_names + [partition_name]

    def _body(*args):
        operands = list(args)
        if partition_name is not None:
            operands.append(partition_id_tensor())
        outs = _bass_exec_p.bind(
            *operands,
            out_avals=tuple(out_avals),
            in_names=tuple(all_names),
            out_names=tuple(out_names),
            lowering_input_output_aliases=(),
            sim_require_finite=True,
            sim_require_nnan=True,
            nc=nc,
        )
        return tuple(outs)

    devices = jax.devices()[:N_CORES]
    mesh = Mesh(np.asarray(devices), ("core",))
    nspecs = n_params + len(out_names)
    fn = jax.jit(
        shard_map(
            _body,
            mesh=mesh,
            in_specs=(PartitionSpec("core"),) * nspecs,
            out_specs=(PartitionSpec("core"),) * len(out_names),
            check_rep=False,
        ),
        keep_unused=True,
    )
    sharding = NamedSharding(mesh, PartitionSpec("core"))
    return fn, in_names, zero_outs, sharding


_STAGED = {}


def _get_runner(dt_name, reps):
    key = (dt_name, reps)
    if key not in _RUNNER_CACHE:
        nc = _get_nc(dt_name, reps)
        _RUNNER_CACHE[key] = _make_runner(nc)
    return _RUNNER_CACHE[key]


def _stage(dt_name, in_maps, in_names, zero_outs, sharding, fresh=False):
    import jax

    if fresh or dt_name not in _STAGED:
        concat_in = [
            jax.device_put(
                np.concatenate([m[name] for m in in_maps], axis=0), sharding
            )
            for name in in_names
        ]
        concat_zero = [
            jax.device_put(
                np.zeros((N_CORES * z.shape[0], *z.shape[1:]), z.dtype), sharding
            )
            for z in zero_outs
        ]
        jax.block_until_ready(concat_in)
        _STAGED[dt_name] = (concat_in, concat_zero)
    return _STAGED[dt_name]


def _run_n(fn, concat_in, concat_zero, n):
    import time

    import jax

    t0 = time.perf_counter()
    last = None
    for _ in range(n):
        last = fn(*concat_in, *concat_zero)
    jax.block_until_ready(last)
    return time.perf_counter() - t0


def time_kernel_ns(n_iter=24, reps=9, rounds=5):
    """Differential HW time per kernel invocation, in ns.

    Times N pipelined executions of the reps=1 and reps=R NEFFs, interleaved
    (A/B alternating, min over rounds) so axon per-call dispatch drift
    (~4 ms/call, +-0.5 ms over minutes) cancels out of the slope."""
    import jax

    assert _LAST_IN_MAPS is not None, "call kernel() first"
    dt_name = os.environ.get("K_DTYPE", "float16")
    runners = {}
    for r in (1, reps):
        fn, in_names, zero_outs, sharding = _get_runner(dt_name, r)
        ci, cz = _stage(dt_name, _LAST_IN_MAPS, in_names, zero_outs, sharding)
        jax.block_until_ready(fn(*ci, *cz))  # compile + warm
        jax.block_until_ready(fn(*ci, *cz))
        runners[r] = (fn, ci, cz)
    t1 = tR = float("inf")
    for _ in range(rounds):
        t1 = min(t1, _run_n(*runners[1], n_iter))
        tR = min(tR, _run_n(*runners[reps], n_iter))
    per_rep = (tR - t1) / (n_iter * (reps - 1))
    if os.environ.get("K_DEBUG"):
        print(
            f"timing: T1={t1 / n_iter * 1e6:.1f} us/call, "
            f"T{reps}={tR / n_iter * 1e6:.1f} us/call, "
            f"diff/rep={per_rep * 1e6:.1f} us"
        )
    return per_rep * 1e9

